# revision 37
# baseline (speedup 1.0000x reference)
"""Trainium2 Bass kernel for nn_ClipCluLoss (clip-cluster loss).

Math (collapsed form of the reference):
    w[b,t]  = 1 / max(||x[b,t,:]||_2, 1e-12)
    s[b,d]  = sum_t w[b,t] * x[b,t,d]          (= T * mean_rep[b,d])
    loss    = T - (1/(B*T)) * sum_b ||s[b]||^2

Sharding: data-parallel over B across 8 NeuronCores (128 samples/core).
Each core returns q[p] = ||s_p||^2 as a [128,1] tensor; the host sums and
does the scalar epilogue.

Per-core structure (x viewed as [4096 rows=(b,t), 1024 d], 32 chunks of
128 rows; whole bf16 shard resident in SBUF, all input DMAs issued
up-front):
  POOL : only SWDGE cast-DMAs f32 HBM -> bf16 SBUF (fp32 matmul on TRN2
         is a 2-pass LOW_HIGH emulation ~4x slower than bf16, so the
         matmul path is bf16; norms/accumulations stay f32). gpsimd does
         nothing else - SWDGE descriptor refill shares the Q7, and any
         compute there starves the DMA stream.
  DVE  : ss = sum_d x^2 for 2 chunks/quad (fused scalar_tensor_tensor)
  ACT  : ss for the other 2 chunks/quad (Square + accum_out in PSUM)
  ACT  : wp = sqrt(ss);  DVE: wp = 1/max(wp, eps)
  DVE  : A[k%NA][:, {4(k-NA), 4k}] = [0 | mask01] * wp  (one strided
         tensor_scalar per chunk builds the block-sparse bf16 lhsT and
         clears the stale block)
  PE   : S[:, :512] += A^T x ; S[:, 512:] += A^T x   (f32 PSUM accum)
  epilogue: DVE copies S to SBUF, fused square+reduce -> q[128,1] -> out.

Raw Bass (manual semaphores): this container's walrus rejects
Tile-generated multi-wait sync and the TENSOR_TENSOR_REDUCE ISA op.
Each input DMA gets its own semaphore: a shared counter with +16 per DMA
is NOT completion-ordered across DMAs (16 SDMA engines increment
independently), which produced data races under 8-core HBM contention.
"""

import sys
from contextlib import ExitStack

import numpy as np

for _p in ("/opt/trn_rl_repo",):
    if _p not in sys.path:
        sys.path.insert(0, _p)

import concourse.bass as bass
from concourse import mybir
from concourse.bass_utils import run_bass_kernel_spmd

B, T, D = 1024, 32, 1024
N_CORES = 8
BS = B // N_CORES            # samples per core
P = 128                      # SBUF partitions
ROWS = BS * T                # 4096 rows of (b,t) per core
NCHUNK = ROWS // P           # 32 chunks of 128 rows
QUADS = NCHUNK // 4          # 4 chunks per quad
EPS = 1e-12

NS = 4                       # ss/wp rotation depth (quads)
NA = 8                       # A (lhsT) buffers (chunks)

F32 = mybir.dt.float32
BF16 = mybir.dt.bfloat16
ALU = mybir.AluOpType
ACTF = mybir.ActivationFunctionType

# DMA units: (first_chunk, n_chunks). Chunk-granular at head and tail so
# the compute pipeline ramps/drains with ~512 KiB latency, 2 MiB quads
# in the middle. Each unit completes on its own semaphore.
DMA_UNITS = (
    [(h, 1) for h in range(4)]
    + [(4 * q, 4) for q in range(1, QUADS - 1)]
    + [(NCHUNK - 4 + h, 1) for h in range(4)]
)
_CHUNK_UNIT = {}
for _u, (_c0, _n) in enumerate(DMA_UNITS):
    for _c in range(_c0, _c0 + _n):
        _CHUNK_UNIT[_c] = _u
assert len(_CHUNK_UNIT) == NCHUNK


def build_bass(debug: bool = False) -> bass.Bass:
    nc = bass.Bass(trn_type="TRN2")
    x_h = nc.declare_dram_parameter("x", [BS, T, D], F32, isOutput=False)
    out_h = nc.declare_dram_parameter("out", [P, 2], F32, isOutput=True)
    dbg_h = None
    if debug:
        dbg_h = nc.declare_dram_parameter("dbg", [P, 1024 + 32 + 8 * P], F32,
                                          isOutput=True)
    x_flat = x_h[:, :, :].flatten_outer_dims()      # [4096, 1024]

    ctx = ExitStack()
    with ctx:
        xb = [
            ctx.enter_context(nc.sbuf_tensor(f"xb{i}", [P, 4 * D], BF16))
            for i in range(QUADS)
        ]
        a_t = [
            ctx.enter_context(nc.sbuf_tensor(f"a_t{i}", [P, P], BF16))
            for i in range(NA)
        ]
        scr_d = ctx.enter_context(nc.sbuf_tensor("scr_d", [P, D], BF16))
        scr_a = ctx.enter_context(nc.sbuf_tensor("scr_a", [P, D], BF16))
        ss_d = ctx.enter_context(nc.sbuf_tensor("ss_d", [P, 2 * NS], F32))
        wp = [
            ctx.enter_context(nc.sbuf_tensor(f"wp{i}", [P, 4], F32))
            for i in range(NS)
        ]
        mask01 = ctx.enter_context(nc.sbuf_tensor("mask01", [P, 4], BF16))
        qa = ctx.enter_context(nc.sbuf_tensor("qa", [P, 1], F32))
        qb = ctx.enter_context(nc.sbuf_tensor("qb", [P, 1], F32))
        qab = ctx.enter_context(nc.sbuf_tensor("qab", [P, 2], F32))
        sepo = ctx.enter_context(nc.sbuf_tensor("sepo", [P, 512], F32))
        dum = ctx.enter_context(nc.sbuf_tensor("dum", [P, 1], F32))
        dbg_t = None
        if debug:
            dbg_t = ctx.enter_context(
                nc.sbuf_tensor("dbgt", [P, 1024 + 32 + 8 * P], F32)
            )

        s_ps = ctx.enter_context(nc.psum_tensor([P, 1024], F32))
        ss_a = ctx.enter_context(nc.psum_tensor([P, 2 * NS], F32))

        dsem = [
            ctx.enter_context(nc.semaphore(f"dsem{u}"))
            for u in range(len(DMA_UNITS))
        ]
        odma_sem = ctx.enter_context(nc.semaphore("odma_sem"))
        ss_sem = ctx.enter_context(nc.semaphore("ss_sem"))      # DVE STTs /quad
        sqrt_sem = ctx.enter_context(nc.semaphore("sqrt_sem"))  # ACT sqrt /quad
        w_sem = ctx.enter_context(nc.semaphore("w_sem"))        # DVE recip /quad
        a_sem = ctx.enter_context(nc.semaphore("a_sem"))        # POOL masks /quad
        mm_sem = ctx.enter_context(nc.semaphore("mm_sem"))      # PE /quad
        fin_sem = ctx.enter_context(nc.semaphore("fin_sem"))
        block = ctx.enter_context(nc.Block())

        def xb_chunk(k):
            """bf16 SBUF view of chunk k: [128, 1024]."""
            q, h = k // 4, k % 4
            return xb[q][:, D * h : D * (h + 1)]

        def wait_chunk(eng, k):
            eng.wait_ge(dsem[_CHUNK_UNIT[k]], 16)

        @block.gpsimd
        def _(g):
            def issue_unit(u):
                c0, n = DMA_UNITS[u]
                q = c0 // 4
                src = x_flat[128 * c0 : 128 * (c0 + n), :]
                if n > 1:
                    src = src.rearrange("(h p) d -> p h d", p=P)
                    dst = xb[q][:, :].rearrange("p (h d) -> p h d", h=4)
                else:
                    dst = xb_chunk(c0)
                g.dma_start(out=dst, in_=src).then_inc(dsem[u], 16)

            # enough units up-front to keep SDMA fed; buffers written once,
            # so no WAR waits anywhere on the input stream
            n_pre = 8
            for u in range(n_pre):
                issue_unit(u)
            for i in range(NA):
                g.memset(a_t[i][:, :], 0.0)
            g.memset(mask01[:, :], 0.0)
            for j in range(4):
                g.memset(mask01[32 * j : 32 * (j + 1), j : j + 1], 1.0)
            # block-sparse lhsT build (DVE's tensor_scalar mangles bf16 at
            # column offsets; gpsimd is proven correct here), interleaved
            # with the remaining DMA issues
            for q in range(QUADS):
                g.wait_ge(w_sem, q + 1)
                if q >= 2:
                    g.wait_ge(mm_sem, q - 1)  # WAR: PE done with quad q-2's A
                for h in range(4):
                    k = 4 * q + h
                    if k >= NA:
                        g.memset(
                            a_t[k % NA][:, 4 * (k - NA) : 4 * (k - NA) + 4], 0.0
                        )
                    ins = g.tensor_scalar_mul(
                        out=a_t[k % NA][:, 4 * k : 4 * k + 4],
                        in0=mask01[:, :],
                        scalar1=wp[q % NS][:, h : h + 1],
                    )
                ins.then_inc(a_sem, 1)
                for u in range(n_pre + 2 * q, min(n_pre + 2 * q + 2, len(DMA_UNITS))):
                    issue_unit(u)
            # merge the two per-bank accumulators for one contiguous out-DMA
            g.wait_ge(fin_sem, 2)
            g.tensor_copy(out=qab[:, 0:1], in_=qa[:, :])
            g.tensor_copy(out=qab[:, 1:2], in_=qb[:, :]).then_inc(fin_sem, 1)

        @block.vector
        def _(v):
            def wmask(q):
                c = q % NS
                v.wait_ge(sqrt_sem, q + 1)
                v.tensor_scalar_max(out=wp[c][:, :], in0=wp[c][:, :], scalar1=EPS)
                v.reciprocal(out=wp[c][:, :], in_=wp[c][:, :]).then_inc(w_sem, 1)

            for q in range(QUADS):
                for h in (0, 1):
                    k = 4 * q + h
                    wait_chunk(v, k)
                    ins = v.scalar_tensor_tensor(
                        out=scr_d[:, :],
                        in0=xb_chunk(k),
                        scalar=1.0,
                        in1=xb_chunk(k),
                        op0=ALU.mult,
                        op1=ALU.mult,
                        accum_out=ss_d[:, 2 * (q % NS) + h : 2 * (q % NS) + h + 1],
                    )
                    if h == 1:
                        ins.then_inc(ss_sem, 1)
                if q >= 1:
                    wmask(q - 1)
            wmask(QUADS - 1)

            if debug:
                v.wait_ge(fin_sem, 3)
                v.tensor_copy(out=dbg_t[:, 0:1024], in_=s_ps[:, :])
                v.tensor_copy(out=dbg_t[:, 1024:1032], in_=ss_d[:, :])
                v.tensor_copy(out=dbg_t[:, 1032:1040], in_=ss_a[:, :])
                for i in range(NS):
                    v.tensor_copy(out=dbg_t[:, 1040 + 4 * i : 1044 + 4 * i],
                                  in_=wp[i][:, :])
                for i in range(NA):
                    ins = v.tensor_copy(
                        out=dbg_t[:, 1056 + P * i : 1056 + P * (i + 1)],
                        in_=a_t[i][:, :],
                    )
                ins.then_inc(fin_sem, 1)

        @block.scalar
        def _(s):
            # trigger the sqrt ACT table load during the first DMA
            s.sqrt(out=dum[:, :], in_=dum[:, :])

            def sqrtstep(q):
                c = q % NS
                s.wait_ge(ss_sem, q + 1)
                if q >= NS:
                    s.wait_ge(a_sem, q - NS + 1)  # WAR: wp[c] readers done
                s.sqrt(out=wp[c][:, 0:2], in_=ss_d[:, 2 * c : 2 * c + 2])
                s.sqrt(out=wp[c][:, 2:4], in_=ss_a[:, 2 * c : 2 * c + 2]).then_inc(
                    sqrt_sem, 1
                )

            for q in range(QUADS):
                for h in (2, 3):
                    k = 4 * q + h
                    wait_chunk(s, k)
                    s.activation(
                        out=scr_a[:, :],
                        in_=xb_chunk(k),
                        func=ACTF.Square,
                        accum_out=ss_a[:, 2 * (q % NS) + h - 2 : 2 * (q % NS) + h - 1],
                    )
                if q >= 1:
                    sqrtstep(q - 1)
            sqrtstep(QUADS - 1)

            # epilogue: q[p] = sum_f S[p, f]^2, one ACT Square+accum per bank
            s.wait_ge(mm_sem, QUADS)
            s.activation(
                out=sepo[:, :], in_=s_ps[:, 0:512], func=ACTF.Square,
                accum_out=qa[:, :],
            ).then_inc(fin_sem, 1)
            s.activation(
                out=sepo[:, :], in_=s_ps[:, 512:1024], func=ACTF.Square,
                accum_out=qb[:, :],
            ).then_inc(fin_sem, 1)

        @block.tensor
        def _(t):
            for q in range(QUADS):
                t.wait_ge(a_sem, q + 1)
                for h in range(4):
                    k = 4 * q + h
                    start = k == 0
                    stop = k == NCHUNK - 1
                    t.matmul(
                        s_ps[:, 0:512],
                        a_t[k % NA][:, :],
                        xb_chunk(k)[:, 0:512],
                        start=start,
                        stop=stop,
                    )
                    ins = t.matmul(
                        s_ps[:, 512:1024],
                        a_t[k % NA][:, :],
                        xb_chunk(k)[:, 512:1024],
                        start=start,
                        stop=stop,
                    )
                ins.then_inc(mm_sem, 1)

        @block.sync
        def _(sp):
            sp.wait_ge(fin_sem, 3)
            sp.dma_start(out=out_h[:, :], in_=qab[:, :]).then_inc(odma_sem, 16)
            if debug:
                sp.wait_ge(fin_sem, 4)
                sp.dma_start(out=dbg_h[:, :], in_=dbg_t[:, :]).then_inc(
                    odma_sem, 16
                )

    return nc


_NC_CACHE: dict = {}


def _get_nc() -> bass.Bass:
    if "nc" not in _NC_CACHE:
        _NC_CACHE["nc"] = build_bass()
    return _NC_CACHE["nc"]


def run_cores(x: np.ndarray, **spmd_kwargs):
    """Run the SPMD kernel on 8 cores. Returns (partials, BassKernelResults)."""
    nc = _get_nc()
    in_maps = [
        {"x": np.ascontiguousarray(x[c * BS : (c + 1) * BS])}
        for c in range(N_CORES)
    ]
    res = run_bass_kernel_spmd(nc, in_maps, core_ids=list(range(N_CORES)),
                               **spmd_kwargs)
    partials = [float(r["out"].astype(np.float64).sum())
                for r in res.results]
    return partials, res


def kernel(inputs: np.ndarray) -> np.ndarray:
    x = np.ascontiguousarray(np.asarray(inputs, dtype=np.float32))
    assert x.shape == (B, T, D), x.shape
    partials, _ = run_cores(x)
    loss = np.float64(T) - np.float64(sum(partials)) / (B * T)
    return np.array(loss, dtype=np.float32)


# revision 44
# speedup vs baseline: 1.0322x; 1.0322x over previous
"""Trainium2 Bass kernel for nn_ClipCluLoss (clip-cluster loss).

Math (collapsed form of the reference):
    w[b,t]  = 1 / max(||x[b,t,:]||_2, 1e-12)
    s[b,d]  = sum_t w[b,t] * x[b,t,d]          (= T * mean_rep[b,d])
    loss    = T - (1/(B*T)) * sum_b ||s[b]||^2

Sharding: data-parallel over B across 8 NeuronCores (128 samples/core).
Each core returns q[p] = ||s_p||^2 as a [128,1] tensor; the host sums and
does the scalar epilogue.

Per-core structure (x viewed as [4096 rows=(b,t), 1024 d], 32 chunks of
128 rows; whole bf16 shard resident in SBUF, all input DMAs issued
up-front):
  POOL : only SWDGE cast-DMAs f32 HBM -> bf16 SBUF (fp32 matmul on TRN2
         is a 2-pass LOW_HIGH emulation ~4x slower than bf16, so the
         matmul path is bf16; norms/accumulations stay f32). gpsimd does
         nothing else - SWDGE descriptor refill shares the Q7, and any
         compute there starves the DMA stream.
  DVE  : ss = sum_d x^2 for 2 chunks/quad (fused scalar_tensor_tensor)
  ACT  : ss for the other 2 chunks/quad (Square + accum_out in PSUM)
  ACT  : wp = sqrt(ss);  DVE: wp = 1/max(wp, eps)
  DVE  : A[k%NA][:, {4(k-NA), 4k}] = [0 | mask01] * wp  (one strided
         tensor_scalar per chunk builds the block-sparse bf16 lhsT and
         clears the stale block)
  PE   : S[:, :512] += A^T x ; S[:, 512:] += A^T x   (f32 PSUM accum)
  epilogue: DVE copies S to SBUF, fused square+reduce -> q[128,1] -> out.

Raw Bass (manual semaphores): this container's walrus rejects
Tile-generated multi-wait sync and the TENSOR_TENSOR_REDUCE ISA op.
Each input DMA gets its own semaphore: a shared counter with +16 per DMA
is NOT completion-ordered across DMAs (16 SDMA engines increment
independently), which produced data races under 8-core HBM contention.
"""

import sys
from contextlib import ExitStack

import numpy as np

for _p in ("/opt/trn_rl_repo",):
    if _p not in sys.path:
        sys.path.insert(0, _p)

import concourse.bass as bass
from concourse import mybir
from concourse.bass_utils import run_bass_kernel_spmd

B, T, D = 1024, 32, 1024
N_CORES = 8
BS = B // N_CORES            # samples per core
P = 128                      # SBUF partitions
ROWS = BS * T                # 4096 rows of (b,t) per core
NCHUNK = ROWS // P           # 32 chunks of 128 rows
QUADS = NCHUNK // 4          # 4 chunks per quad
EPS = 1e-12

NS = 4                       # ss/wp rotation depth (quads)
NA = 8                       # A (lhsT) buffers (chunks)

F32 = mybir.dt.float32
BF16 = mybir.dt.bfloat16
ALU = mybir.AluOpType
ACTF = mybir.ActivationFunctionType

# DMA units: (first_chunk, n_chunks). Chunk-granular at head and tail so
# the compute pipeline ramps/drains with ~512 KiB latency, 2 MiB quads
# in the middle. Each unit completes on its own semaphore.
DMA_UNITS = (
    [(h, 1) for h in range(4)]
    + [(4 * q, 4) for q in range(1, QUADS - 1)]
    + [(NCHUNK - 4 + h, 1) for h in range(4)]
)
_CHUNK_UNIT = {}
for _u, (_c0, _n) in enumerate(DMA_UNITS):
    for _c in range(_c0, _c0 + _n):
        _CHUNK_UNIT[_c] = _u
assert len(_CHUNK_UNIT) == NCHUNK


def build_bass(debug: bool = False) -> bass.Bass:
    nc = bass.Bass(trn_type="TRN2", enable_partition_id=False)
    x_h = nc.declare_dram_parameter("x", [BS, T, D], F32, isOutput=False)
    out_h = nc.declare_dram_parameter("out", [P, 2], F32, isOutput=True)
    dbg_h = None
    if debug:
        dbg_h = nc.declare_dram_parameter("dbg", [P, 1024 + 32 + 8 * P], F32,
                                          isOutput=True)
    x_flat = x_h[:, :, :].flatten_outer_dims()      # [4096, 1024]

    ctx = ExitStack()
    with ctx:
        xb = [
            ctx.enter_context(nc.sbuf_tensor(f"xb{i}", [P, 4 * D], BF16))
            for i in range(QUADS)
        ]
        a_t = [
            ctx.enter_context(nc.sbuf_tensor(f"a_t{i}", [P, P], BF16))
            for i in range(NA)
        ]
        scr_d = ctx.enter_context(nc.sbuf_tensor("scr_d", [P, D], BF16))
        scr_a = ctx.enter_context(nc.sbuf_tensor("scr_a", [P, D], BF16))
        ss_d = ctx.enter_context(nc.sbuf_tensor("ss_d", [P, 2 * NS], F32))
        wp = [
            ctx.enter_context(nc.sbuf_tensor(f"wp{i}", [P, 4], F32))
            for i in range(NS)
        ]
        mask01 = ctx.enter_context(nc.sbuf_tensor("mask01", [P, 4], BF16))
        qa = ctx.enter_context(nc.sbuf_tensor("qa", [P, 1], F32))
        qb = ctx.enter_context(nc.sbuf_tensor("qb", [P, 1], F32))
        qab = ctx.enter_context(nc.sbuf_tensor("qab", [P, 2], F32))
        sepo = ctx.enter_context(nc.sbuf_tensor("sepo", [P, 512], F32))
        dum = ctx.enter_context(nc.sbuf_tensor("dum", [P, 1], F32))
        dbg_t = None
        if debug:
            dbg_t = ctx.enter_context(
                nc.sbuf_tensor("dbgt", [P, 1024 + 32 + 8 * P], F32)
            )

        s_ps = ctx.enter_context(nc.psum_tensor([P, 1024], F32))
        ss_a = ctx.enter_context(nc.psum_tensor([P, 2 * NS], F32))

        dsem = [
            ctx.enter_context(nc.semaphore(f"dsem{u}"))
            for u in range(len(DMA_UNITS))
        ]
        odma_sem = ctx.enter_context(nc.semaphore("odma_sem"))
        ss_sem = ctx.enter_context(nc.semaphore("ss_sem"))      # DVE STTs /quad
        sqrt_sem = ctx.enter_context(nc.semaphore("sqrt_sem"))  # ACT sqrt /quad
        w_sem = ctx.enter_context(nc.semaphore("w_sem"))        # DVE recip /quad
        a_sem = ctx.enter_context(nc.semaphore("a_sem"))        # POOL masks /quad
        mm_sem = ctx.enter_context(nc.semaphore("mm_sem"))      # PE /quad
        fin_sem = ctx.enter_context(nc.semaphore("fin_sem"))
        # chunk-granular sems for the last quad's pipelined tail
        td_sem = ctx.enter_context(nc.semaphore("td_sem"))      # DVE tail STTs
        st_sem = ctx.enter_context(nc.semaphore("st_sem"))      # ACT tail sqrts
        wt_sem = ctx.enter_context(nc.semaphore("wt_sem"))      # DVE tail recips
        at_sem = ctx.enter_context(nc.semaphore("at_sem"))      # POOL tail masks
        block = ctx.enter_context(nc.Block())

        def xb_chunk(k):
            """bf16 SBUF view of chunk k: [128, 1024]."""
            q, h = k // 4, k % 4
            return xb[q][:, D * h : D * (h + 1)]

        def wait_chunk(eng, k):
            eng.wait_ge(dsem[_CHUNK_UNIT[k]], 16)

        @block.gpsimd
        def _(g):
            def issue_unit(u):
                c0, n = DMA_UNITS[u]
                q = c0 // 4
                src = x_flat[128 * c0 : 128 * (c0 + n), :]
                if n > 1:
                    src = src.rearrange("(h p) d -> p h d", p=P)
                    dst = xb[q][:, :].rearrange("p (h d) -> p h d", h=4)
                else:
                    dst = xb_chunk(c0)
                g.dma_start(out=dst, in_=src).then_inc(dsem[u], 16)

            # enough units up-front to keep SDMA fed; buffers written once,
            # so no WAR waits anywhere on the input stream
            n_pre = 8
            for u in range(n_pre):
                issue_unit(u)
            for i in range(NA):
                g.memset(a_t[i][:, :], 0.0)
            g.memset(mask01[:, :], 0.0)
            for j in range(4):
                g.memset(mask01[32 * j : 32 * (j + 1), j : j + 1], 1.0)
            # block-sparse lhsT build (DVE's tensor_scalar mangles bf16 at
            # column offsets; gpsimd is proven correct here), interleaved
            # with the remaining DMA issues
            def maskop(k, wcol):
                if k >= NA:
                    g.memset(
                        a_t[k % NA][:, 4 * (k - NA) : 4 * (k - NA) + 4], 0.0
                    )
                return g.tensor_scalar_mul(
                    out=a_t[k % NA][:, 4 * k : 4 * k + 4],
                    in0=mask01[:, :],
                    scalar1=wcol,
                )

            for q in range(QUADS - 1):
                g.wait_ge(w_sem, q + 1)
                if q >= 2:
                    g.wait_ge(mm_sem, q - 1)  # WAR: PE done with quad q-2's A
                for h in range(4):
                    ins = maskop(4 * q + h, wp[q % NS][:, h : h + 1])
                ins.then_inc(a_sem, 1)
                for u in range(n_pre + 2 * q, min(n_pre + 2 * q + 2, len(DMA_UNITS))):
                    issue_unit(u)
            # pipelined tail: per-chunk masks for the last quad
            qt = QUADS - 1
            g.wait_ge(mm_sem, qt - 1)
            for h in range(4):
                g.wait_ge(wt_sem, h + 1)
                maskop(4 * qt + h, wp[qt % NS][:, h : h + 1]).then_inc(at_sem, 1)
            # merge the two per-bank accumulators for one contiguous out-DMA
            g.wait_ge(fin_sem, 2)
            g.tensor_copy(out=qab[:, 0:1], in_=qa[:, :])
            g.tensor_copy(out=qab[:, 1:2], in_=qb[:, :]).then_inc(fin_sem, 1)

        @block.vector
        def _(v):
            def wmask(q):
                c = q % NS
                v.wait_ge(sqrt_sem, q + 1)
                v.tensor_scalar_max(out=wp[c][:, :], in0=wp[c][:, :], scalar1=EPS)
                v.reciprocal(out=wp[c][:, :], in_=wp[c][:, :]).then_inc(w_sem, 1)

            def stt(k, col):
                wait_chunk(v, k)
                return v.scalar_tensor_tensor(
                    out=scr_d[:, :],
                    in0=xb_chunk(k),
                    scalar=1.0,
                    in1=xb_chunk(k),
                    op0=ALU.mult,
                    op1=ALU.mult,
                    accum_out=ss_d[:, col : col + 1],
                )

            for q in range(QUADS - 1):
                for h in (0, 1):
                    ins = stt(4 * q + h, 2 * (q % NS) + h)
                    if h == 1:
                        ins.then_inc(ss_sem, 1)
                if q >= 1:
                    wmask(q - 1)
            # pipelined tail (last quad): per-chunk STT/recip chains
            qt = QUADS - 1
            ct = qt % NS
            stt(4 * qt, 2 * ct).then_inc(td_sem, 1)
            stt(4 * qt + 1, 2 * ct + 1).then_inc(td_sem, 1)
            wmask(qt - 1)
            for h in range(4):
                v.wait_ge(st_sem, h + 1)
                v.tensor_scalar_max(
                    out=wp[ct][:, h : h + 1], in0=wp[ct][:, h : h + 1], scalar1=EPS
                )
                v.reciprocal(
                    out=wp[ct][:, h : h + 1], in_=wp[ct][:, h : h + 1]
                ).then_inc(wt_sem, 1)

            if debug:
                v.wait_ge(fin_sem, 3)
                v.tensor_copy(out=dbg_t[:, 0:1024], in_=s_ps[:, :])
                v.tensor_copy(out=dbg_t[:, 1024:1032], in_=ss_d[:, :])
                v.tensor_copy(out=dbg_t[:, 1032:1040], in_=ss_a[:, :])
                for i in range(NS):
                    v.tensor_copy(out=dbg_t[:, 1040 + 4 * i : 1044 + 4 * i],
                                  in_=wp[i][:, :])
                for i in range(NA):
                    ins = v.tensor_copy(
                        out=dbg_t[:, 1056 + P * i : 1056 + P * (i + 1)],
                        in_=a_t[i][:, :],
                    )
                ins.then_inc(fin_sem, 1)

        @block.scalar
        def _(s):
            # trigger the sqrt ACT table load during the first DMA
            s.sqrt(out=dum[:, :], in_=dum[:, :])

            def sqrtstep(q):
                c = q % NS
                s.wait_ge(ss_sem, q + 1)
                if q >= NS:
                    s.wait_ge(a_sem, q - NS + 1)  # WAR: wp[c] readers done
                s.sqrt(out=wp[c][:, 0:2], in_=ss_d[:, 2 * c : 2 * c + 2])
                s.sqrt(out=wp[c][:, 2:4], in_=ss_a[:, 2 * c : 2 * c + 2]).then_inc(
                    sqrt_sem, 1
                )

            for q in range(QUADS - 1):
                for h in (2, 3):
                    k = 4 * q + h
                    wait_chunk(s, k)
                    s.activation(
                        out=scr_a[:, :],
                        in_=xb_chunk(k),
                        func=ACTF.Square,
                        accum_out=ss_a[:, 2 * (q % NS) + h - 2 : 2 * (q % NS) + h - 1],
                    )
                if q >= 1:
                    sqrtstep(q - 1)
            sqrtstep(QUADS - 2)
            # pipelined tail (last quad): per-chunk sqrt as each ss arrives
            qt = QUADS - 1
            ct = qt % NS
            s.wait_ge(a_sem, qt - NS + 1)  # WAR: wp[ct] readers done
            for h in range(4):
                if h < 2:
                    s.wait_ge(td_sem, h + 1)
                    src = ss_d[:, 2 * ct + h : 2 * ct + h + 1]
                else:
                    wait_chunk(s, 4 * qt + h)
                    s.activation(
                        out=scr_a[:, :],
                        in_=xb_chunk(4 * qt + h),
                        func=ACTF.Square,
                        accum_out=ss_a[:, 2 * ct + h - 2 : 2 * ct + h - 1],
                    )
                    src = ss_a[:, 2 * ct + h - 2 : 2 * ct + h - 1]
                s.sqrt(out=wp[ct][:, h : h + 1], in_=src).then_inc(st_sem, 1)

            # epilogue: q[p] = sum_f S[p, f]^2, one ACT Square+accum per bank
            s.wait_ge(mm_sem, QUADS)
            s.activation(
                out=sepo[:, :], in_=s_ps[:, 0:512], func=ACTF.Square,
                accum_out=qa[:, :],
            ).then_inc(fin_sem, 1)
            s.activation(
                out=sepo[:, :], in_=s_ps[:, 512:1024], func=ACTF.Square,
                accum_out=qb[:, :],
            ).then_inc(fin_sem, 1)

        @block.tensor
        def _(t):
            def mmpair(k):
                start = k == 0
                stop = k == NCHUNK - 1
                t.matmul(
                    s_ps[:, 0:512],
                    a_t[k % NA][:, :],
                    xb_chunk(k)[:, 0:512],
                    start=start,
                    stop=stop,
                )
                return t.matmul(
                    s_ps[:, 512:1024],
                    a_t[k % NA][:, :],
                    xb_chunk(k)[:, 512:1024],
                    start=start,
                    stop=stop,
                )

            for q in range(QUADS - 1):
                t.wait_ge(a_sem, q + 1)
                for h in range(4):
                    ins = mmpair(4 * q + h)
                ins.then_inc(mm_sem, 1)
            # pipelined tail: per-chunk matmuls for the last quad
            for h in range(4):
                t.wait_ge(at_sem, h + 1)
                ins = mmpair(4 * (QUADS - 1) + h)
            ins.then_inc(mm_sem, 1)

        @block.sync
        def _(sp):
            sp.wait_ge(fin_sem, 3)
            sp.dma_start(out=out_h[:, :], in_=qab[:, :]).then_inc(odma_sem, 16)
            if debug:
                sp.wait_ge(fin_sem, 4)
                sp.dma_start(out=dbg_h[:, :], in_=dbg_t[:, :]).then_inc(
                    odma_sem, 16
                )

    return nc


_NC_CACHE: dict = {}


def _get_nc() -> bass.Bass:
    if "nc" not in _NC_CACHE:
        _NC_CACHE["nc"] = build_bass()
    return _NC_CACHE["nc"]


def run_cores(x: np.ndarray, **spmd_kwargs):
    """Run the SPMD kernel on 8 cores. Returns (partials, BassKernelResults)."""
    nc = _get_nc()
    in_maps = [
        {"x": np.ascontiguousarray(x[c * BS : (c + 1) * BS])}
        for c in range(N_CORES)
    ]
    res = run_bass_kernel_spmd(nc, in_maps, core_ids=list(range(N_CORES)),
                               **spmd_kwargs)
    partials = [float(r["out"].astype(np.float64).sum())
                for r in res.results]
    return partials, res


def kernel(inputs: np.ndarray) -> np.ndarray:
    x = np.ascontiguousarray(np.asarray(inputs, dtype=np.float32))
    assert x.shape == (B, T, D), x.shape
    partials, _ = run_cores(x)
    loss = np.float64(T) - np.float64(sum(partials)) / (B * T)
    return np.array(loss, dtype=np.float32)
